# revision 7
# baseline (speedup 1.0000x reference)
"""Local attention (window 33) Trainium2 Bass kernel, 8-core sequence-parallel.

Layout: B=2, T=4096, C=768, H=12, D=64. Core c handles batch c//4,
token chunk [1024*(c%4), 1024*(c%4+1)) with 16-token halos. No collectives.

Per-core program:
  qT = (Wq x^T + bq) * D^-0.5   [C, 1024]  (transposed layout, 2 heads / 128-row tile)
  kT = Wk x^T + bk              [C, 1056]
  v  = x Wv^T                   [1056, C]  (natural layout; bv folded in post-attention)
  per (q-tile 128, head): s = qT^T kT window [128,160]; +mask; exp (+row sums);
    normalize; PE-transpose -> aT; oT[d,tq] = v^T aT + bv
  yT = Wo ocat^T + bo           [C, 1024] -> host transposes/assembles
"""

import sys

for _p in ("/opt/trn_rl_repo",):
    if _p not in sys.path:
        sys.path.insert(0, _p)

import numpy as np

from concourse import bacc, bass, masks, mybir, tile
from concourse.bass_utils import run_bass_kernel_spmd

B, T, C = 2, 4096, 768
H, D = 12, 64
HALF_W = 16
T_LOC = 1024          # tokens per core
T_HALO = T_LOC + 2 * HALF_W   # 1056
NK = C // 128         # 6 contraction tiles
NQT = T_LOC // 128    # 8 query tiles per core
WIN = 160             # key window per 128-query tile
F32 = mybir.dt.float32
AF = mybir.ActivationFunctionType

_CACHE = {}


def _build_program():
    if "nc" in _CACHE:
        return _CACHE["nc"]

    nc = bacc.Bacc("TRN2", target_bir_lowering=False, debug=False, num_devices=8)

    xT_d = nc.dram_tensor("xT", [C, T_HALO], F32, kind="ExternalInput").ap()
    wqT_d = nc.dram_tensor("wqT", [C, C], F32, kind="ExternalInput").ap()
    wkT_d = nc.dram_tensor("wkT", [C, C], F32, kind="ExternalInput").ap()
    wvT_d = nc.dram_tensor("wvT", [C, C], F32, kind="ExternalInput").ap()
    woT_d = nc.dram_tensor("woT", [C, C], F32, kind="ExternalInput").ap()
    bq_d = nc.dram_tensor("bq", [C, 1], F32, kind="ExternalInput").ap()
    bk_d = nc.dram_tensor("bk", [C, 1], F32, kind="ExternalInput").ap()
    bv_d = nc.dram_tensor("bv", [C, 1], F32, kind="ExternalInput").ap()
    bo_d = nc.dram_tensor("bo", [C, 1], F32, kind="ExternalInput").ap()
    mask_d = nc.dram_tensor("mask", [NQT, 128, WIN], F32, kind="ExternalInput").ap()
    yT_d = nc.dram_tensor("yT", [C, T_LOC], F32, kind="ExternalOutput").ap()

    with tile.TileContext(nc, trace_sim=False) as tc:
        _emit(tc, xT_d, wqT_d, wkT_d, wvT_d, woT_d, bq_d, bk_d, bv_d, bo_d,
              mask_d, yT_d)

    nc.compile()
    _CACHE["nc"] = nc
    return nc


def _emit(tc, xT_d, wqT_d, wkT_d, wvT_d, woT_d, bq_d, bk_d, bv_d, bo_d,
          mask_d, yT_d):
    nc = tc.nc
    import contextlib
    ctx = contextlib.ExitStack()

    const = ctx.enter_context(tc.tile_pool(name="const", bufs=1))
    xp = ctx.enter_context(tc.tile_pool(name="xp", bufs=1))
    wp = ctx.enter_context(tc.tile_pool(name="wp", bufs=2 * NK))
    qp = ctx.enter_context(tc.tile_pool(name="qp", bufs=1))
    kp = ctx.enter_context(tc.tile_pool(name="kp", bufs=1))
    vp = ctx.enter_context(tc.tile_pool(name="vp", bufs=1))
    op = ctx.enter_context(tc.tile_pool(name="op", bufs=1))
    mp = ctx.enter_context(tc.tile_pool(name="mp", bufs=2))
    ap_pool = ctx.enter_context(tc.tile_pool(name="ap", bufs=3))
    atp = ctx.enter_context(tc.tile_pool(name="atp", bufs=3))
    sp = ctx.enter_context(tc.tile_pool(name="sp", bufs=4))
    yp = ctx.enter_context(tc.tile_pool(name="yp", bufs=3))

    ps_proj = ctx.enter_context(tc.tile_pool(name="ps_proj", bufs=2, space="PSUM"))
    ps_s = ctx.enter_context(tc.tile_pool(name="ps_s", bufs=2, space="PSUM"))
    ps_tr = ctx.enter_context(tc.tile_pool(name="ps_tr", bufs=2, space="PSUM"))
    ps_av = ctx.enter_context(tc.tile_pool(name="ps_av", bufs=2, space="PSUM"))

    ident = const.tile([128, 128], F32)
    masks.make_identity(nc, ident[:])

    def load_bias(dram, pfx):
        ts = [const.tile([128, 1], F32, tag=f"{pfx}{i}", name=f"{pfx}{i}")
              for i in range(NK)]
        for i in range(NK):
            nc.sync.dma_start(ts[i][:], dram[128 * i:128 * (i + 1), :])
        return ts

    bq_t = load_bias(bq_d, "bq")
    bk_t = load_bias(bk_d, "bk")
    bv_t = load_bias(bv_d, "bv")
    bo_t = load_bias(bo_d, "bo")

    xT = [xp.tile([128, T_HALO], F32, tag=f"x{i}", name=f"x{i}") for i in range(NK)]
    for i in range(NK):
        nc.sync.dma_start(xT[i][:], xT_d[128 * i:128 * (i + 1), :])

    def load_w(dram, tag):
        ts = []
        for i in range(NK):
            t = wp.tile([128, C], F32)
            nc.sync.dma_start(t[:], dram[128 * i:128 * (i + 1), :])
            ts.append(t)
        return ts

    # ---- q/k projections (transposed layout) ----
    qT = [qp.tile([128, T_LOC], F32, tag=f"q{i}", name=f"q{i}") for i in range(NK)]
    kT = [kp.tile([128, T_HALO], F32, tag=f"k{i}", name=f"k{i}") for i in range(NK)]

    wq = load_w(wqT_d, "wq")
    for ot in range(NK):
        for lo, wdt in ((0, 512), (512, 512)):
            ps = ps_proj.tile([128, 512], F32)
            for kt in range(NK):
                nc.tensor.matmul(
                    ps[:, 0:wdt],
                    lhsT=wq[kt][:, 128 * ot:128 * (ot + 1)],
                    rhs=xT[kt][:, HALF_W + lo:HALF_W + lo + wdt],
                    start=(kt == 0), stop=(kt == NK - 1),
                )
            nc.scalar.activation(qT[ot][:, lo:lo + wdt], ps[:, 0:wdt],
                                 AF.Identity, bias=bq_t[ot][:])

    wk = load_w(wkT_d, "wk")
    for ot in range(NK):
        for lo, wdt in ((0, 512), (512, 512), (1024, 32)):
            ps = ps_proj.tile([128, 512], F32)
            for kt in range(NK):
                nc.tensor.matmul(
                    ps[:, 0:wdt],
                    lhsT=wk[kt][:, 128 * ot:128 * (ot + 1)],
                    rhs=xT[kt][:, lo:lo + wdt],
                    start=(kt == 0), stop=(kt == NK - 1),
                )
            nc.scalar.activation(kT[ot][:, lo:lo + wdt], ps[:, 0:wdt],
                                 AF.Identity, bias=bk_t[ot][:])

    # ---- v projection (natural layout, no bias) ----
    v = [vp.tile([128, C], F32, tag=f"v{i}", name=f"v{i}") for i in range(9)]
    wv = load_w(wvT_d, "wv")
    for tt in range(9):
        rows = 128 if tt < 8 else T_HALO - 8 * 128
        for half in range(2):
            ps = ps_proj.tile([128, 512], F32)
            for kt in range(NK):
                nc.tensor.matmul(
                    ps[0:rows, 0:384],
                    lhsT=xT[kt][:, 128 * tt:128 * tt + rows],
                    rhs=wv[kt][:, 384 * half:384 * (half + 1)],
                    start=(kt == 0), stop=(kt == NK - 1),
                )
            nc.scalar.activation(v[tt][0:rows, 384 * half:384 * (half + 1)],
                                 ps[0:rows, 0:384], AF.Copy)

    # ---- attention ----
    # oT holds head pairs: head h lives at partitions (h%2)*64 of tile h//2.
    oT = [op.tile([128, T_LOC], F32, tag=f"o{j}", name=f"o{j}") for j in range(H // 2)]
    for qt in range(NQT):
        mt = mp.tile([128, WIN], F32)
        nc.sync.dma_start(mt[:], mask_d[qt])
        for hp in range(H // 2):
            pso = ps_av.tile([128, 128], F32)
            for sub in range(2):
                h = 2 * hp + sub
                pr = slice(64 * sub, 64 * sub + 64)

                pss = ps_s.tile([128, WIN], F32)
                nc.tensor.matmul(
                    pss[:],
                    lhsT=qT[hp][pr, 128 * qt:128 * (qt + 1)],
                    rhs=kT[hp][pr, 128 * qt:128 * qt + WIN],
                    start=True, stop=True,
                )
                a = ap_pool.tile([128, WIN], F32, tag="a")
                nc.vector.tensor_add(a[:], pss[:], mt[:])
                ssum = sp.tile([128, 1], F32, tag="ssum")
                nc.scalar.activation(a[:], a[:], AF.Exp, accum_out=ssum[:])
                rs = sp.tile([128, 1], F32, tag="rs")
                nc.vector.reciprocal(rs[:], ssum[:])
                nc.vector.tensor_scalar_mul(a[:], a[:], rs[:])

                pst = ps_tr.tile([128, 256], F32)
                nc.tensor.transpose(pst[:, 0:128], a[:, 0:128], ident[:])
                nc.tensor.transpose(pst[0:32, 128:256], a[:, 128:WIN], ident[:])
                aT = atp.tile([128, 256], F32, tag="aT")
                nc.vector.tensor_copy(aT[:, 0:128], pst[:, 0:128])
                nc.vector.tensor_copy(aT[0:32, 128:256], pst[0:32, 128:256])

                nc.tensor.matmul(pso[pr, :], lhsT=v[qt][:, 64 * h:64 * (h + 1)],
                                 rhs=aT[:, 0:128], start=True, stop=False,
                                 tile_position=(0, 64 * sub))
                nc.tensor.matmul(pso[pr, :],
                                 lhsT=v[qt + 1][0:32, 64 * h:64 * (h + 1)],
                                 rhs=aT[0:32, 128:256], start=False, stop=True,
                                 tile_position=(0, 64 * sub))
            nc.scalar.activation(oT[hp][:, 128 * qt:128 * (qt + 1)], pso[:],
                                 AF.Identity, bias=bv_t[hp][:])

    # ---- output projection ----
    wo = load_w(woT_d, "wo")
    for ot in range(NK):
        for nb in range(2):
            ps = ps_proj.tile([128, 512], F32)
            for j in range(H // 2):
                nc.tensor.matmul(
                    ps[:],
                    lhsT=wo[j][:, 128 * ot:128 * (ot + 1)],
                    rhs=oT[j][:, 512 * nb:512 * (nb + 1)],
                    start=(j == 0), stop=(j == H // 2 - 1),
                )
            yt = yp.tile([128, 512], F32, tag="y")
            nc.scalar.activation(yt[:], ps[:], AF.Identity, bias=bo_t[ot][:])
            nc.sync.dma_start(yT_d[128 * ot:128 * (ot + 1), 512 * nb:512 * (nb + 1)],
                              yt[:])

    ctx.close()


def _host_prep(x, Wq, bq, Wk, bk, Wv, bv, Wo, bo):
    x = np.asarray(x, np.float32)
    scale = np.float32(D ** -0.5)
    wqT = np.ascontiguousarray(np.asarray(Wq, np.float32).T) * scale
    wkT = np.ascontiguousarray(np.asarray(Wk, np.float32).T)
    wvT = np.ascontiguousarray(np.asarray(Wv, np.float32).T)
    woT = np.ascontiguousarray(np.asarray(Wo, np.float32).T)
    bqs = (np.asarray(bq, np.float32) * scale).reshape(C, 1)
    bks = np.asarray(bk, np.float32).reshape(C, 1).copy()
    bvs = np.asarray(bv, np.float32).reshape(C, 1).copy()
    bos = np.asarray(bo, np.float32).reshape(C, 1).copy()

    in_maps = []
    for c in range(8):
        b, chunk = c // 4, c % 4
        t0 = T_LOC * chunk
        xt = np.zeros((C, T_HALO), np.float32)
        h0 = HALF_W if chunk == 0 else 0
        h1 = T_HALO - HALF_W if chunk == 3 else T_HALO
        xt[:, h0:h1] = x[b, t0 - HALF_W + h0:t0 - HALF_W + h1, :].T

        # mask[qt, i, j]: query global t0+128qt+i, key global t0-16+128qt+j
        qt = np.arange(NQT)[:, None, None]
        i = np.arange(128)[None, :, None]
        j = np.arange(WIN)[None, None, :]
        gk = t0 - HALF_W + 128 * qt + j
        valid = (j >= i) & (j <= i + 2 * HALF_W) & (gk >= 0) & (gk < T)
        mask = np.where(valid, np.float32(0.0), np.float32(-1e30))

        in_maps.append({
            "xT": xt, "wqT": wqT, "wkT": wkT, "wvT": wvT, "woT": woT,
            "bq": bqs, "bk": bks, "bv": bvs, "bo": bos,
            "mask": np.ascontiguousarray(mask, np.float32),
        })
    return in_maps


def kernel(x, Wq, bq, Wk, bk, Wv, bv, Wo, bo, _trace=False, _results=None):
    in_maps = _host_prep(x, Wq, bq, Wk, bk, Wv, bv, Wo, bo)
    nc = _build_program()
    res = run_bass_kernel_spmd(nc, in_maps, list(range(8)), trace=_trace)
    if _results is not None:
        _results.append(res)
    out = np.empty((B, T, C), np.float32)
    for c in range(8):
        b, chunk = c // 4, c % 4
        out[b, T_LOC * chunk:T_LOC * (chunk + 1), :] = res.results[c]["yT"].T
    return out


# revision 9
# speedup vs baseline: 4458.9027x; 4458.9027x over previous
"""Local attention (window 33) Trainium2 Bass kernel, 8-core sequence-parallel.

Layout: B=2, T=4096, C=768, H=12, D=64. Core c handles batch c//4,
token chunk [1024*(c%4), 1024*(c%4+1)) with 16-token halos. No collectives.

Per-core program:
  qT = (Wq x^T + bq) * D^-0.5   [C, 1024]  (transposed layout, 2 heads / 128-row tile)
  kT = Wk x^T + bk              [C, 1056]
  v  = x Wv^T                   [1056, C]  (natural layout; bv folded in post-attention)
  per (q-tile 128, head): s = qT^T kT window [128,160]; +mask; exp (+row sums);
    normalize; PE-transpose -> aT; oT[d,tq] = v^T aT + bv
  yT = Wo ocat^T + bo           [C, 1024] -> host transposes/assembles
"""

import sys

for _p in ("/opt/trn_rl_repo",):
    if _p not in sys.path:
        sys.path.insert(0, _p)

import numpy as np

from concourse import bacc, bass, masks, mybir, tile
from concourse.bass_utils import run_bass_kernel_spmd

B, T, C = 2, 4096, 768
H, D = 12, 64
HALF_W = 16
T_LOC = 1024          # tokens per core
T_HALO = T_LOC + 2 * HALF_W   # 1056
NK = C // 128         # 6 contraction tiles
NQT = T_LOC // 128    # 8 query tiles per core
WIN = 160             # key window per 128-query tile
F32 = mybir.dt.float32
F32R = mybir.dt.float32r
AF = mybir.ActivationFunctionType

_CACHE = {}


def _build_program():
    if "nc" in _CACHE:
        return _CACHE["nc"]

    nc = bacc.Bacc("TRN2", target_bir_lowering=False, debug=False, num_devices=8)

    xT_d = nc.dram_tensor("xT", [C, T_HALO], F32, kind="ExternalInput").ap()
    wqT_d = nc.dram_tensor("wqT", [C, C], F32, kind="ExternalInput").ap()
    wkT_d = nc.dram_tensor("wkT", [C, C], F32, kind="ExternalInput").ap()
    wvT_d = nc.dram_tensor("wvT", [C, C], F32, kind="ExternalInput").ap()
    woT_d = nc.dram_tensor("woT", [C, C], F32, kind="ExternalInput").ap()
    bq_d = nc.dram_tensor("bq", [C, 1], F32, kind="ExternalInput").ap()
    bk_d = nc.dram_tensor("bk", [C, 1], F32, kind="ExternalInput").ap()
    bv_d = nc.dram_tensor("bv", [C, 1], F32, kind="ExternalInput").ap()
    bo_d = nc.dram_tensor("bo", [C, 1], F32, kind="ExternalInput").ap()
    mask_d = nc.dram_tensor("mask", [NQT, 128, WIN], F32, kind="ExternalInput").ap()
    yT_d = nc.dram_tensor("yT", [C, T_LOC], F32, kind="ExternalOutput").ap()

    with tile.TileContext(nc, trace_sim=False) as tc:
        _emit(tc, xT_d, wqT_d, wkT_d, wvT_d, woT_d, bq_d, bk_d, bv_d, bo_d,
              mask_d, yT_d)

    nc.compile()
    _CACHE["nc"] = nc
    return nc


def _emit(tc, xT_d, wqT_d, wkT_d, wvT_d, woT_d, bq_d, bk_d, bv_d, bo_d,
          mask_d, yT_d):
    nc = tc.nc
    import contextlib
    ctx = contextlib.ExitStack()

    const = ctx.enter_context(tc.tile_pool(name="const", bufs=1))
    xp = ctx.enter_context(tc.tile_pool(name="xp", bufs=1))
    wp = ctx.enter_context(tc.tile_pool(name="wp", bufs=2 * NK))
    qp = ctx.enter_context(tc.tile_pool(name="qp", bufs=1))
    kp = ctx.enter_context(tc.tile_pool(name="kp", bufs=1))
    vp = ctx.enter_context(tc.tile_pool(name="vp", bufs=1))
    op = ctx.enter_context(tc.tile_pool(name="op", bufs=1))
    mp = ctx.enter_context(tc.tile_pool(name="mp", bufs=2))
    ap_pool = ctx.enter_context(tc.tile_pool(name="ap", bufs=3))
    atp = ctx.enter_context(tc.tile_pool(name="atp", bufs=3))
    sp = ctx.enter_context(tc.tile_pool(name="sp", bufs=4))
    yp = ctx.enter_context(tc.tile_pool(name="yp", bufs=3))

    ps_proj = ctx.enter_context(tc.tile_pool(name="ps_proj", bufs=2, space="PSUM"))
    ps_s = ctx.enter_context(tc.tile_pool(name="ps_s", bufs=2, space="PSUM"))
    ps_tr = ctx.enter_context(tc.tile_pool(name="ps_tr", bufs=2, space="PSUM"))
    ps_av = ctx.enter_context(tc.tile_pool(name="ps_av", bufs=2, space="PSUM"))

    ident = const.tile([128, 128], F32)
    masks.make_identity(nc, ident[:])

    def load_bias(dram, pfx):
        ts = [const.tile([128, 1], F32, tag=f"{pfx}{i}", name=f"{pfx}{i}")
              for i in range(NK)]
        for i in range(NK):
            nc.sync.dma_start(ts[i][:], dram[128 * i:128 * (i + 1), :])
        return ts

    bq_t = load_bias(bq_d, "bq")
    bk_t = load_bias(bk_d, "bk")
    bv_t = load_bias(bv_d, "bv")
    bo_t = load_bias(bo_d, "bo")

    xT = [xp.tile([128, T_HALO], F32, tag=f"x{i}", name=f"x{i}") for i in range(NK)]
    for i in range(NK):
        nc.sync.dma_start(xT[i][:], xT_d[128 * i:128 * (i + 1), :])

    def load_w(dram, tag):
        ts = []
        for i in range(NK):
            t = wp.tile([128, C], F32)
            nc.sync.dma_start(t[:], dram[128 * i:128 * (i + 1), :])
            ts.append(t)
        return ts

    # ---- q/k projections (transposed layout) ----
    qT = [qp.tile([128, T_LOC], F32, tag=f"q{i}", name=f"q{i}") for i in range(NK)]
    kT = [kp.tile([128, T_HALO], F32, tag=f"k{i}", name=f"k{i}") for i in range(NK)]

    wq = load_w(wqT_d, "wq")
    for ot in range(NK):
        for lo, wdt in ((0, 512), (512, 512)):
            ps = ps_proj.tile([128, 512], F32)
            for kt in range(NK):
                nc.tensor.matmul(
                    ps[:, 0:wdt],
                    lhsT=wq[kt][:, 128 * ot:128 * (ot + 1)],
                    rhs=xT[kt][:, HALF_W + lo:HALF_W + lo + wdt],
                    start=(kt == 0), stop=(kt == NK - 1),
                )
            nc.scalar.activation(qT[ot][:, lo:lo + wdt], ps[:, 0:wdt],
                                 AF.Identity, bias=bq_t[ot][:])

    wk = load_w(wkT_d, "wk")
    for ot in range(NK):
        for lo, wdt in ((0, 512), (512, 512), (1024, 32)):
            ps = ps_proj.tile([128, 512], F32)
            for kt in range(NK):
                nc.tensor.matmul(
                    ps[:, 0:wdt],
                    lhsT=wk[kt][:, 128 * ot:128 * (ot + 1)],
                    rhs=xT[kt][:, lo:lo + wdt],
                    start=(kt == 0), stop=(kt == NK - 1),
                )
            nc.scalar.activation(kT[ot][:, lo:lo + wdt], ps[:, 0:wdt],
                                 AF.Identity, bias=bk_t[ot][:])

    # ---- v projection (natural layout, no bias) ----
    v = [vp.tile([128, C], F32, tag=f"v{i}", name=f"v{i}") for i in range(9)]
    wv = load_w(wvT_d, "wv")
    for tt in range(9):
        rows = 128 if tt < 8 else T_HALO - 8 * 128
        for half in range(2):
            ps = ps_proj.tile([128, 512], F32)
            for kt in range(NK):
                nc.tensor.matmul(
                    ps[0:rows, 0:384],
                    lhsT=xT[kt][:, 128 * tt:128 * tt + rows],
                    rhs=wv[kt][:, 384 * half:384 * (half + 1)],
                    start=(kt == 0), stop=(kt == NK - 1),
                )
            nc.scalar.activation(v[tt][0:rows, 384 * half:384 * (half + 1)],
                                 ps[0:rows, 0:384], AF.Copy)

    # ---- attention ----
    # oT holds head pairs: head h lives at partitions (h%2)*64 of tile h//2.
    oT = [op.tile([128, T_LOC], F32, tag=f"o{j}", name=f"o{j}") for j in range(H // 2)]
    for qt in range(NQT):
        mt = mp.tile([128, WIN], F32)
        nc.sync.dma_start(mt[:], mask_d[qt])
        for hp in range(H // 2):
            pso = ps_av.tile([128, 128], F32)
            for sub in range(2):
                h = 2 * hp + sub
                pr = slice(64 * sub, 64 * sub + 64)

                pss = ps_s.tile([128, WIN], F32)
                nc.tensor.matmul(
                    pss[:],
                    lhsT=qT[hp][pr, 128 * qt:128 * (qt + 1)],
                    rhs=kT[hp][pr, 128 * qt:128 * qt + WIN],
                    start=True, stop=True,
                )
                a = ap_pool.tile([128, WIN], F32, tag="a")
                nc.vector.tensor_add(a[:], pss[:], mt[:])
                ssum = sp.tile([128, 1], F32, tag="ssum")
                nc.scalar.activation(a[:], a[:], AF.Exp, accum_out=ssum[:])
                rs = sp.tile([128, 1], F32, tag="rs")
                nc.vector.reciprocal(rs[:], ssum[:])
                nc.vector.tensor_scalar_mul(a[:], a[:], rs[:])

                pst = ps_tr.tile([128, 256], F32)
                nc.tensor.transpose(pst[:, 0:128], a[:, 0:128], ident[:])
                nc.tensor.transpose(pst[0:32, 128:256], a[:, 128:WIN], ident[:])
                aT = atp.tile([128, 256], F32, tag="aT")
                nc.vector.tensor_copy(aT[:, 0:128], pst[:, 0:128])
                nc.vector.tensor_copy(aT[0:32, 128:256], pst[0:32, 128:256])

                nc.tensor.matmul(pso[pr, :], lhsT=v[qt][:, 64 * h:64 * (h + 1)],
                                 rhs=aT[:, 0:128], start=True, stop=False,
                                 tile_position=(0, 64 * sub))
                nc.tensor.matmul(pso[pr, :],
                                 lhsT=v[qt + 1][0:32, 64 * h:64 * (h + 1)],
                                 rhs=aT[0:32, 128:256], start=False, stop=True,
                                 tile_position=(0, 64 * sub))
            nc.scalar.activation(oT[hp][:, 128 * qt:128 * (qt + 1)], pso[:],
                                 AF.Identity, bias=bv_t[hp][:])

    # ---- output projection ----
    wo = load_w(woT_d, "wo")
    for ot in range(NK):
        for nb in range(2):
            ps = ps_proj.tile([128, 512], F32)
            for j in range(H // 2):
                nc.tensor.matmul(
                    ps[:],
                    lhsT=wo[j][:, 128 * ot:128 * (ot + 1)],
                    rhs=oT[j][:, 512 * nb:512 * (nb + 1)],
                    start=(j == 0), stop=(j == H // 2 - 1),
                )
            yt = yp.tile([128, 512], F32, tag="y")
            nc.scalar.activation(yt[:], ps[:], AF.Identity, bias=bo_t[ot][:])
            nc.sync.dma_start(yT_d[128 * ot:128 * (ot + 1), 512 * nb:512 * (nb + 1)],
                              yt[:])

    ctx.close()


def _host_prep(x, Wq, bq, Wk, bk, Wv, bv, Wo, bo):
    x = np.asarray(x, np.float32)
    scale = np.float32(D ** -0.5)
    wqT = np.ascontiguousarray(np.asarray(Wq, np.float32).T) * scale
    wkT = np.ascontiguousarray(np.asarray(Wk, np.float32).T)
    wvT = np.ascontiguousarray(np.asarray(Wv, np.float32).T)
    woT = np.ascontiguousarray(np.asarray(Wo, np.float32).T)
    bqs = (np.asarray(bq, np.float32) * scale).reshape(C, 1)
    bks = np.asarray(bk, np.float32).reshape(C, 1).copy()
    bvs = np.asarray(bv, np.float32).reshape(C, 1).copy()
    bos = np.asarray(bo, np.float32).reshape(C, 1).copy()

    in_maps = []
    for c in range(8):
        b, chunk = c // 4, c % 4
        t0 = T_LOC * chunk
        xt = np.zeros((C, T_HALO), np.float32)
        h0 = HALF_W if chunk == 0 else 0
        h1 = T_HALO - HALF_W if chunk == 3 else T_HALO
        xt[:, h0:h1] = x[b, t0 - HALF_W + h0:t0 - HALF_W + h1, :].T

        # mask[qt, i, j]: query global t0+128qt+i, key global t0-16+128qt+j
        qt = np.arange(NQT)[:, None, None]
        i = np.arange(128)[None, :, None]
        j = np.arange(WIN)[None, None, :]
        gk = t0 - HALF_W + 128 * qt + j
        valid = (j >= i) & (j <= i + 2 * HALF_W) & (gk >= 0) & (gk < T)
        mask = np.where(valid, np.float32(0.0), np.float32(-1e30))

        in_maps.append({
            "xT": xt, "wqT": wqT, "wkT": wkT, "wvT": wvT, "woT": woT,
            "bq": bqs, "bk": bks, "bv": bvs, "bo": bos,
            "mask": np.ascontiguousarray(mask, np.float32),
        })
    return in_maps


def kernel(x, Wq, bq, Wk, bk, Wv, bv, Wo, bo, _trace=False, _results=None):
    in_maps = _host_prep(x, Wq, bq, Wk, bk, Wv, bv, Wo, bo)
    nc = _build_program()
    res = run_bass_kernel_spmd(nc, in_maps, list(range(8)), trace=_trace)
    if _results is not None:
        _results.append(res)
    out = np.empty((B, T, C), np.float32)
    for c in range(8):
        b, chunk = c // 4, c % 4
        out[b, T_LOC * chunk:T_LOC * (chunk + 1), :] = res.results[c]["yT"].T
    return out
